# revision 42
# baseline (speedup 1.0000x reference)
"""ALiBi causal attention layer on 8 TRN2 NeuronCores.

Sharding: data parallel on batch (B=2) x tensor parallel on heads (16 -> 4
groups of 4).  Core c = 4*b + g computes, for batch element b, the STRIDED
head set {g, 4+g, 8+g, 12+g} end to end: QKV projections (column-sharded),
causal ALiBi attention, and the row-sharded output projection.  The host
sums the 4 partial outputs per batch element (the tensor-parallel
all-reduce) and adds the output bias.  The striding makes head slot j hold
global heads {4j..4j+3} on every core, so each slot's ALiBi slope range is
uniform and the SPMD-shared graph can window steep slots' attention: slot 0
(slopes >= 0.25) looks back only 120 positions, slot 1 (>= 0.0625) 480 --
skipped k-tiles contribute < 1e-11 to the softmax.

Device kernel (all matmuls in float32r, ~1e-4 rel err, fp32 PSUM accum):
  - x arrives host-transposed with a ones row: xt [1025, 2048]; projection
    biases ride in an augmented contraction row of each weight matrix.
  - K^T lives in per-head [128, 2048] tiles: head data at its native
    partition parity (even head rows 0:64, odd rows 64:128), the ALiBi
    rank-2 rows (slope*8*k, ones) adjacent, remaining rows zeroed.  Q^T
    uses matching per-(head, q-block) [128, 512] tiles with rows
    (ones, -slope*8*q).  S^T = K_aug^T.T @ Q_aug then exp() directly on
    ACT with scale=1/8 (max-free softmax: scores are bounded), so S^T
    already includes the ALiBi bias.
  - Causality: k-tiles fully above the diagonal are skipped; diagonal
    tiles are zero-filled post-exp with gpsimd.affine_select.
  - V carries a ones column per head, so the PV matmul yields O^T plus the
    softmax denominators; O^T *= 1/den via DVE reciprocal + PE broadcast.
"""
import math

import ml_dtypes
import numpy as np

BF = ml_dtypes.bfloat16

import concourse.bass as bass
import concourse.tile as tile
from concourse import mybir, bacc
from concourse.bass_utils import run_bass_kernel_spmd

F32 = mybir.dt.float32
F32R = mybir.dt.float32r
BF16 = mybir.dt.bfloat16

B, T, C, H = 2, 2048, 1024, 16
D = C // H            # 64 head dim
NCORES = 8
HG = 4                # heads per core
CG = HG * D           # 256 channels per core
VW = HG * (D + 1)     # 260: V with a ones column per head
QB = 512              # q block width
KTW = 128             # k tile width
NQB = T // QB         # 4
NKT = T // KTW        # 16
NCH = C // 128        # 8 contraction chunks


def _slopes(n):
    def p2(m):
        start = 2 ** (-(2 ** -(math.log2(m) - 3)))
        return [start * start**i for i in range(m)]
    if math.log2(n).is_integer():
        return p2(n)
    c = 2 ** math.floor(math.log2(n))
    return p2(c) + _slopes(2 * c)[0::2][: n - c]


def _build():
    nc = bacc.Bacc()
    xt = nc.declare_dram_parameter("xt", [C + 1, T], BF16, isOutput=False)
    wq = nc.declare_dram_parameter("wq", [C + 1, CG], BF16, isOutput=False)
    wk = nc.declare_dram_parameter("wk", [C + 1, CG], BF16, isOutput=False)
    wv = nc.declare_dram_parameter("wv", [C + 1, VW], BF16, isOutput=False)
    wo = nc.declare_dram_parameter("wo", [CG, C], BF16, isOutput=False)
    hka = nc.declare_dram_parameter("hka", [HG, 2, T], F32R, isOutput=False)
    hqa = nc.declare_dram_parameter("hqa", [HG, 2, T], F32R, isOutput=False)
    stair = nc.declare_dram_parameter("stair", [128, 640], F32R, isOutput=False)
    ident = nc.declare_dram_parameter("ident", [128, 128], F32R, isOutput=False)
    hbias = nc.declare_dram_parameter("hbias", [128, 128], F32, isOutput=False)
    y = nc.declare_dram_parameter("y", [T, C], BF16, isOutput=True)

    EXP = mybir.ActivationFunctionType.Exp
    CPY = mybir.ActivationFunctionType.Copy

    with tile.TileContext(nc) as tc, \
         nc.allow_low_precision(reason="fp32r compute"):
        with tc.tile_pool(name="const", bufs=1) as cp, \
             tc.tile_pool(name="xtp", bufs=20) as xtp, \
             tc.tile_pool(name="qap", bufs=8) as qap, \
             tc.tile_pool(name="otp", bufs=4) as otp, \
             tc.tile_pool(name="ptp", bufs=6) as ptp, \
             tc.tile_pool(name="yp", bufs=2) as ypool, \
             tc.tile_pool(name="misc", bufs=2) as mp, \
             tc.tile_pool(name="ps", bufs=6, space="PSUM") as psp, \
             tc.tile_pool(name="po", bufs=2, space="PSUM") as pop:

            # ---- constants: weights, aug rows, zero fill ----
            # DMA emission order matters for time-to-first-matmul: wq and
            # the first x block go first so the Q projection can start while
            # the rest of the constants stream in.
            wq_sb = [cp.tile([128, CG], BF16, tag=f"wq{c}", name=f"wq{c}") for c in range(NCH)]
            wk_sb = [cp.tile([128, CG], BF16, tag=f"wk{c}", name=f"wk{c}") for c in range(NCH)]
            wv_sb = [cp.tile([128, VW], BF16, tag=f"wv{c}", name=f"wv{c}") for c in range(NCH)]
            wo_sb = [cp.tile([128, C], BF16, tag=f"wo{c}", name=f"wo{c}") for c in range(2)]
            wqb = cp.tile([1, CG], BF16, tag="wqb")
            wkb = cp.tile([1, CG], BF16, tag="wkb")
            wvb = cp.tile([1, VW], BF16, tag="wvb")
            ones_sb = cp.tile([1, QB], BF16, tag="ones")
            ones_fr = cp.tile([1, 128], F32R, tag="ones_fr")
            ones32 = cp.tile([1, 128], F32, tag="ones32")
            nc.vector.memset(ones32[:], 1.0)
            nc.vector.tensor_copy(ones_fr[:], ones32[:])
            xts0 = []
            for c in range(NCH):
                nc.sync.dma_start(wq_sb[c][:], wq[128 * c:128 * (c + 1), :])
                xtt = xtp.tile([128, QB], BF16, tag="xt", name=f"xt0_{c}")
                nc.sync.dma_start(xtt[:], xt[128 * c:128 * (c + 1), 0:QB])
                xts0.append(xtt)
            nc.sync.dma_start(wqb[:], wq[C:C + 1, :])
            nc.sync.dma_start(ones_sb[:], xt[C:C + 1, 0:QB])

            for c in range(NCH):
                nc.sync.dma_start(wk_sb[c][:], wk[128 * c:128 * (c + 1), :])
            nc.sync.dma_start(wkb[:], wk[C:C + 1, :])

            zf = cp.tile([128, QB], F32, tag="zf")
            nc.vector.memset(zf[:], 0.0)

            # causal-mask staircase: stair[p, f] = -3000 where f - 128 < p.
            # Accumulating I.T @ stair[:, off:off+W] into a diagonal S tile
            # drives masked (k > q) scores to -3000 pre-exp, so the exp
            # underflows to 0 and no post-exp select is needed.  These (and
            # the ka aug rows) are needed before wv/wo, so they DMA first.
            stair_sb = cp.tile([128, 640], F32R, tag="stair")
            ident_sb = cp.tile([128, 128], F32R, tag="ident")
            hb_sb = cp.tile([128, 128], F32, tag="hb")
            nc.sync.dma_start(stair_sb[:], stair[:])
            nc.sync.dma_start(ident_sb[:], ident[:])
            nc.sync.dma_start(hb_sb[:], hbias[:])

            # Slots 0,1 (steep ALiBi slopes): per-head K^T tiles with the
            # rank-2 aug-row ALiBi.  Even head: data rows 0:64, aug rows
            # 64:66, zeros 66:128.  Odd head: aug 0:2, zeros 2:64, data
            # 64:128.  K aug = (slope8*k, ones).
            # Slots 2,3 (shallow slopes): one packed [128, T] K^T tile, slot2
            # on rows 0:64 and slot3 on rows 64:128; their ALiBi rides the
            # exp as a per-partition ACT bias slope*(k - q0) (the per-q part
            # cancels between softmax numerator and denominator), so the two
            # slots' S matmuls row-tile the PE concurrently.
            kap23 = cp.tile([128, T], F32R, tag="kap23")
            ka = [cp.tile([128, T], F32R, tag=f"ka{h}", name=f"ka{h}") for h in range(2)]
            for h in range(2):
                par = h % 2
                arow = 64 if par == 0 else 0
                # zero the whole non-data half (32-aligned partition base),
                # then the aug-row DMA overwrites its 2 rows
                for blk in range(NQB):
                    sl = slice(QB * blk, QB * (blk + 1))
                    nc.vector.tensor_copy(ka[h][arow:arow + 64, sl],
                                          zf[arow:arow + 64, :])
                nc.sync.dma_start(ka[h][arow:arow + 2, :], hka[h])

            for c in range(NCH):
                nc.sync.dma_start(wv_sb[c][:], wv[128 * c:128 * (c + 1), :])
            nc.sync.dma_start(wvb[:], wv[C:C + 1, :])
            for c in range(2):
                nc.sync.dma_start(wo_sb[c][:], wo[128 * c:128 * (c + 1), :])

            v_sb = [cp.tile([128, VW], F32R, tag=f"v{t}", name=f"v{t}") for t in range(NKT)]

            # ---- fused, software-pipelined per-block loop ----
            def proj(qb):
                """QKV projections for t-block qb; returns the Q tiles."""
                tsl = slice(QB * qb, QB * (qb + 1))
                if qb == 0:
                    xts = xts0
                else:
                    xts = []
                    for c in range(NCH):
                        xtt = xtp.tile([128, QB], BF16, tag="xt",
                                       name=f"xt{qb}_{c}")
                        nc.sync.dma_start(xtt[:],
                                          xt[128 * c:128 * (c + 1), tsl])
                        xts.append(xtt)

                qa_t = []
                for h in range(2):
                    qat = qap.tile([128, QB], F32R, tag="qa",
                                   name=f"qa{qb}_{h}")
                    par = h % 2
                    arow = 64 if par == 0 else 0
                    nc.vector.tensor_copy(qat[arow:arow + 64, :],
                                          zf[arow:arow + 64, :])
                    nc.sync.dma_start(qat[arow:arow + 2, :], hqa[h][:, tsl])
                    qa_t.append(qat)
                q23 = qap.tile([128, QB], F32R, tag="qa",
                               name=f"q23_{qb}")
                qa_t.append(q23)

                for wsb, wb, is_q in ((wq_sb, wqb, True), (wk_sb, wkb, False)):
                    for m in range(2):
                        ps = psp.tile([128, QB], F32, tag="ps")
                        for c in range(NCH):
                            nc.tensor.matmul(
                                ps[:], wsb[c][:, 128 * m:128 * (m + 1)],
                                xts[c][:], start=(c == 0), stop=False,
                                skip_group_check=True)
                        nc.tensor.matmul(
                            ps[:], wb[:, 128 * m:128 * (m + 1)], ones_sb[:],
                            start=False, stop=True, skip_group_check=True)
                        if m == 1:
                            # packed pair: slot2 rows 0:64, slot3 rows
                            # 64:128, exactly the proj PSUM layout
                            if is_q:
                                nc.vector.tensor_copy(q23[:], ps[:])
                            else:
                                nc.vector.tensor_copy(kap23[:, tsl], ps[:])
                            continue
                        for j in range(2):
                            h = 2 * m + j
                            rows = slice(64 * j, 64 * j + 64)
                            if is_q:
                                nc.vector.tensor_copy(qa_t[h][rows, :],
                                                      ps[rows, :])
                            else:
                                nc.vector.tensor_copy(ka[h][rows, tsl],
                                                      ps[rows, :])

                for tt in range(4):
                    kt = 4 * qb + tt
                    psv = psp.tile([128, QB], F32, tag="ps")
                    for c in range(NCH):
                        nc.tensor.matmul(
                            psv[:, 0:VW],
                            xts[c][:, 128 * tt:128 * (tt + 1)], wv_sb[c][:],
                            start=(c == 0), stop=False, skip_group_check=True)
                    nc.tensor.matmul(
                        psv[:, 0:VW], ones_sb[:, 0:128], wvb[:],
                        start=False, stop=True, skip_group_check=True)
                    nc.vector.tensor_copy(v_sb[kt][:], psv[:, 0:VW])
                return qa_t

            qa_next = proj(0)
            for qb in range(NQB):
                qa_t = qa_next
                # attention for this q-block.  Pass A per head is the
                # PE-heavy S/exp/mask/PV chain; pass B (recip -> broadcast
                # -> divide) for head h is emitted after head h+1's pass A
                # so the broadcast matmul never sits at the front of the PE
                # queue waiting on the DVE reciprocal.
                po_t = {}
                ot_t = [otp.tile([128, QB], BF16, tag="ot",
                                 name=f"ot_{qb}_{c}") for c in range(2)]

                # ALiBi windows per head slot: with the strided head
                # assignment, slot j holds global heads {4j..4j+3}; a tile
                # whose every (k, q) pair has slope*(k-q) <= -14 contributes
                # < 1e-4 relative attention mass (well under the 2e-2 rel-err
                # budget).  W_j = 14 / min-slope-in-slot.
                WIN = (56.0, 224.0, 897.0, 1e9)

                # Diagonal k-tile tt (tt = kt - 4*qb) only matters for q
                # columns >= 128*tt, so trim its S/exp/PV to [C_tt, 512).
                # tt=3 keeps 256 cols (f32r needs a >=256 moving dim); its
                # extra cols [256,384) are fully masked by the staircase.
                TRIM = ((0, QB), (128, 384), (256, 256), (256, 256))

                def finish_head(h, po):
                    den = mp.tile([1, QB], F32, tag="den", bufs=2,
                                  name=f"den_{qb}_{h}")
                    nc.vector.tensor_copy(den[:], po[D:D + 1, :])
                    rc32 = mp.tile([1, QB], F32, tag="rc32", bufs=2,
                                   name=f"rc32_{qb}_{h}")
                    nc.vector.reciprocal_approx_fast(rc32[:], den[:])
                    rc = mp.tile([1, QB], F32R, tag="rc", bufs=4,
                                 name=f"rc_{qb}_{h}")
                    nc.vector.tensor_copy(rc[:], rc32[:])
                    po_t[h] = (po, rc)

                def pass_a(h):
                    # diagonal tiles go first so tile tt=0 opens the full
                    # [0,512) PV accumulation region and the head's tail is
                    # short-latency
                    full = [kt for kt in range(4 * qb)
                            if 128 * kt > QB * qb - WIN[h] - 127]
                    kts = list(range(4 * qb, 4 * qb + 4)) + full
                    po = pop.tile([D + 1, QB], F32, tag="po",
                                  name=f"po_{qb}_{h}")
                    for i, kt in enumerate(kts):
                        tt = kt - 4 * qb
                        if tt >= 0:
                            c0, w = TRIM[tt]
                        else:
                            c0, w = 0, QB
                        pss = psp.tile([128, QB], F32, tag="ps")
                        nc.tensor.matmul(
                            pss[:, 0:w], ka[h][:, 128 * kt:128 * (kt + 1)],
                            qa_t[h][:, c0:c0 + w], start=True, stop=(tt < 0),
                            skip_group_check=True)
                        if tt >= 0:
                            # masked (k > q) entries get -3000 pre-exp
                            soff = 0 if tt == 3 else 128
                            nc.tensor.matmul(
                                pss[:, 0:w], ident_sb[:],
                                stair_sb[:, soff:soff + w], start=False,
                                stop=True, skip_group_check=True)
                        pt = ptp.tile([128, QB], F32R, tag="pt")
                        nc.scalar.activation(pt[:, 0:w], pss[:, 0:w], EXP,
                                             bias=0.0, scale=0.125)
                        nc.tensor.matmul(
                            po[:, c0:c0 + w],
                            v_sb[kt][:, 65 * h:65 * (h + 1)], pt[:, 0:w],
                            start=(i == 0), stop=(i == len(kts) - 1),
                            skip_group_check=True)
                    finish_head(h, po)

                def pass_a23():
                    # slots 2,3 share one packed K/Q tile; common k-tiles
                    # issue as two concurrent row-tiled S matmuls
                    full2 = [kt for kt in range(4 * qb)
                             if 128 * kt > QB * qb - WIN[2] - 127]
                    kts = list(range(4 * qb, 4 * qb + 4)) + list(range(4 * qb))
                    po2 = pop.tile([D + 1, QB], F32, tag="po",
                                   name=f"po_{qb}_2")
                    po3 = pop.tile([D + 1, QB], F32, tag="po",
                                   name=f"po_{qb}_3")
                    n2 = 4 + len(full2)
                    n3 = len(kts)
                    i2 = i3 = 0
                    for kt in kts:
                        tt = kt - 4 * qb
                        if tt >= 0:
                            c0, w = TRIM[tt]
                        else:
                            c0, w = 0, QB
                        ktsl = slice(128 * kt, 128 * (kt + 1))
                        has2 = tt >= 0 or kt in full2
                        if has2:
                            pss2 = psp.tile([128, QB], F32, tag="ps")
                            nc.tensor.matmul(
                                pss2[:, 0:w], kap23[0:64, ktsl],
                                qa_t[2][0:64, c0:c0 + w], start=True,
                                stop=(tt < 0), skip_group_check=True,
                                tile_position=(0, 0))
                        pss3 = psp.tile([128, QB], F32, tag="ps")
                        nc.tensor.matmul(
                            pss3[:, 0:w], kap23[64:128, ktsl],
                            qa_t[2][64:128, c0:c0 + w], start=True,
                            stop=(tt < 0), skip_group_check=True,
                            tile_position=(64, 0))
                        if tt >= 0:
                            soff = 0 if tt == 3 else 128
                            nc.tensor.matmul(
                                pss2[:, 0:w], ident_sb[:],
                                stair_sb[:, soff:soff + w], start=False,
                                stop=True, skip_group_check=True)
                            nc.tensor.matmul(
                                pss3[:, 0:w], ident_sb[:],
                                stair_sb[:, soff:soff + w], start=False,
                                stop=True, skip_group_check=True)
                        if has2:
                            bcol = 16 * qb + kt
                            pt2 = ptp.tile([128, QB], F32R, tag="pt")
                            nc.scalar.activation(
                                pt2[:, 0:w], pss2[:, 0:w], EXP,
                                bias=hb_sb[:, bcol:bcol + 1], scale=0.125)
                            nc.tensor.matmul(
                                po2[:, c0:c0 + w],
                                v_sb[kt][:, 65 * 2:65 * 3], pt2[:, 0:w],
                                start=(i2 == 0), stop=(i2 == n2 - 1),
                                skip_group_check=True)
                            i2 += 1
                        bcol = 64 + 16 * qb + kt
                        pt3 = ptp.tile([128, QB], F32R, tag="pt")
                        nc.scalar.activation(
                            pt3[:, 0:w], pss3[:, 0:w], EXP,
                            bias=hb_sb[:, bcol:bcol + 1], scale=0.125)
                        nc.tensor.matmul(
                            po3[:, c0:c0 + w],
                            v_sb[kt][:, 65 * 3:65 * 4], pt3[:, 0:w],
                            start=(i3 == 0), stop=(i3 == n3 - 1),
                            skip_group_check=True)
                        i3 += 1
                    finish_head(3, po3)
                    finish_head(2, po2)

                def pass_b(h):
                    po, rc = po_t.pop(h)
                    pb = psp.tile([D, QB], F32, tag="ps",
                                  name=f"pb_{qb}_{h}")
                    nc.tensor.matmul(pb[:], ones_fr[:, 0:D], rc[:],
                                     start=True, stop=True,
                                     skip_group_check=True)
                    bc = mp.tile([D, QB], F32, tag="bc", bufs=4,
                                 name=f"bc_{qb}_{h}")
                    nc.vector.tensor_copy(bc[:], pb[:])
                    pair = ot_t[h // 2]
                    if h % 2 == 0:
                        nc.vector.tensor_tensor(pair[0:D, :], po[0:D, :],
                                                bc[:],
                                                op=mybir.AluOpType.mult)
                    else:
                        # odd head's O^T lands at partitions 0:64; DVE
                        # cannot shift partitions, so divide into a temp
                        # then DMA it into rows 64:128 of the pair tile
                        tmp = mp.tile([D, QB], BF16, tag="ottmp", bufs=4,
                                      name=f"ottmp_{qb}_{h}")
                        nc.vector.tensor_tensor(tmp[:], po[0:D, :], bc[:],
                                                op=mybir.AluOpType.mult)
                        # scalar HWDGE queue: keeps the Sync queue (which
                        # carries the xt prefetch) free of this hop
                        nc.scalar.dma_start(pair[D:2 * D, :], tmp[:])

                # Slots 2,3 (packed pair) first, then 1, then 0, so the qb's
                # trailing pass_b chain ends on even head 0 (no DMA hop); the
                # output projection starts on pair 1 (ready mid-sequence).
                pass_a23()
                pass_a(1)
                pass_b(3)
                pass_a(0)
                pass_b(2)

                # next q-block's projections are emitted BEFORE the last two
                # pass_b's and the output projection: the PE queue is
                # in-order, so these ready proj matmuls cover the ~3us DVE
                # recip/bcast chains of the trailing heads.
                if qb + 1 < NQB:
                    qa_next = proj(qb + 1)
                pass_b(1)
                pass_b(0)

                # output projection for this t-block (pair 1 first)
                for tt in range(4):
                    t = 4 * qb + tt
                    fsl = slice(128 * tt, 128 * (tt + 1))
                    ysb = ypool.tile([128, C], BF16, tag="y",
                                     name=f"y_{qb}_{tt}")
                    for half in range(2):
                        hsl = slice(QB * half, QB * (half + 1))
                        py = psp.tile([128, QB], F32, tag="ps")
                        for c in (1, 0):
                            nc.tensor.matmul(
                                py[:], ot_t[c][:, fsl], wo_sb[c][:, hsl],
                                start=(c == 1), stop=(c == 0),
                                skip_group_check=True)
                        nc.scalar.activation(ysb[:, hsl], py[:], CPY)
                        # sync queue: emitted after proj(qb+1)'s prefetch
                        # triggers, so these can't block the next q-block
                        nc.sync.dma_start(y[128 * t:128 * (t + 1), hsl],
                                          ysb[:, hsl])
    nc.finalize()
    return nc


_NC_CACHE = None


def _get_nc():
    global _NC_CACHE
    if _NC_CACHE is None:
        _NC_CACHE = _build()
    return _NC_CACHE


def kernel(x, Wq, bq, Wk, bk, Wv, bv, Wo, bo):
    x = np.asarray(x, dtype=np.float32)
    Wq, bq = np.asarray(Wq, np.float32), np.asarray(bq, np.float32)
    Wk, bk = np.asarray(Wk, np.float32), np.asarray(bk, np.float32)
    Wv, bv = np.asarray(Wv, np.float32), np.asarray(bv, np.float32)
    Wo, bo = np.asarray(Wo, np.float32), np.asarray(bo, np.float32)

    slopes = np.asarray(_slopes(H), dtype=np.float32)
    ar = np.arange(T, dtype=np.float32)

    pp, ff = np.meshgrid(np.arange(128), np.arange(640), indexing="ij")
    stair_np = np.where(ff - 128 < pp, -3000.0, 0.0).astype(np.float32)
    ident_np = np.eye(128, dtype=np.float32)

    xts = []
    for b in range(B):
        xa = np.empty((C + 1, T), np.float32)
        xa[:C] = x[b].T
        xa[C] = 1.0
        xts.append(np.ascontiguousarray(xa.astype(BF)))

    pr = np.arange(128, dtype=np.float32)
    shards = []
    for g in range(HG):
        # strided head assignment: core g, slot j <-> global head 4j+g, so
        # each slot's ALiBi slope range is uniform across cores and the
        # (SPMD-shared) graph can window steep slots' attention
        heads = [HG * j + g for j in range(HG)]
        # ACT-bias table for slots 2,3: col = 64*(slot-2) + 16*qb + kt,
        # value[p] = slope * (128*kt + p - 512*qb)
        hb = np.zeros((128, 128), np.float32)
        for sl in (2, 3):
            s = slopes[heads[sl]]
            for qbn in range(4):
                for kt in range(16):
                    col = 64 * (sl - 2) + 16 * qbn + kt
                    hb[:, col] = s * (128.0 * kt + pr - 512.0 * qbn)
        cols = np.concatenate([np.arange(D * h, D * (h + 1)) for h in heads])
        wqa = np.concatenate([Wq[:, cols], bq[None, cols]], axis=0)
        wka = np.concatenate([Wk[:, cols], bk[None, cols]], axis=0)
        wva = np.zeros((C + 1, VW), np.float32)
        for j, h in enumerate(heads):
            hsl = slice(D * h, D * (h + 1))
            wva[:C, 65 * j:65 * j + D] = Wv[:, hsl]
            wva[C, 65 * j:65 * j + D] = bv[hsl]
            wva[C, 65 * j + D] = 1.0
        woa = np.ascontiguousarray(Wo[cols, :])
        hk = np.empty((HG, 2, T), np.float32)
        hq = np.empty((HG, 2, T), np.float32)
        for j, h in enumerate(heads):
            # K rows (k, s8) pair with Q rows (s8, -q): S += s8*(k - q).
            # Integer k/q are exact on the f32r grid and s8 rounds once, so
            # the large terms cancel exactly in the fp32 PSUM accumulator
            # (splitting s8*k / s8*q would round each entry independently
            # and leave O(s8*T*eps) noise in the scores).
            s8 = 8.0 * slopes[h]
            hk[j, 0] = ar
            hk[j, 1] = s8
            hq[j, 0] = s8
            hq[j, 1] = -ar
        shards.append(dict(
            wq=np.ascontiguousarray(wqa.astype(BF)),
            wk=np.ascontiguousarray(wka.astype(BF)),
            wv=wva.astype(BF), wo=np.ascontiguousarray(woa.astype(BF)),
            hka=hk, hqa=hq, stair=stair_np, ident=ident_np, hbias=hb))

    in_maps = []
    for core in range(NCORES):
        b, g = divmod(core, HG)
        m = dict(shards[g])
        m["xt"] = xts[b]
        in_maps.append(m)

    nc = _get_nc()
    res = run_bass_kernel_spmd(nc, in_maps, core_ids=list(range(NCORES)))

    out = np.empty((B, T, C), np.float32)
    for b in range(B):
        acc = res.results[4 * b]["y"].astype(np.float32).copy()
        for g in range(1, HG):
            acc += res.results[4 * b + g]["y"].astype(np.float32)
        out[b] = acc + bo[None, :]
    return out



# revision 43
# speedup vs baseline: 1.0776x; 1.0776x over previous
"""ALiBi causal attention layer on 8 TRN2 NeuronCores.

Sharding: data parallel on batch (B=2) x tensor parallel on heads (16 -> 4
groups of 4).  Core c = 4*b + g computes, for batch element b, the STRIDED
head set {g, 4+g, 8+g, 12+g} end to end: QKV projections (column-sharded),
causal ALiBi attention, and the row-sharded output projection.  The host
sums the 4 partial outputs per batch element (the tensor-parallel
all-reduce) and adds the output bias.  The striding makes head slot j hold
global heads {4j..4j+3} on every core, so each slot's ALiBi slope range is
uniform and the SPMD-shared graph can window steep slots' attention: slot 0
(slopes >= 0.25) looks back only 120 positions, slot 1 (>= 0.0625) 480 --
skipped k-tiles contribute < 1e-11 to the softmax.

Device kernel (all matmuls in float32r, ~1e-4 rel err, fp32 PSUM accum):
  - x arrives host-transposed with a ones row: xt [1025, 2048]; projection
    biases ride in an augmented contraction row of each weight matrix.
  - K^T lives in per-head [128, 2048] tiles: head data at its native
    partition parity (even head rows 0:64, odd rows 64:128), the ALiBi
    rank-2 rows (slope*8*k, ones) adjacent, remaining rows zeroed.  Q^T
    uses matching per-(head, q-block) [128, 512] tiles with rows
    (ones, -slope*8*q).  S^T = K_aug^T.T @ Q_aug then exp() directly on
    ACT with scale=1/8 (max-free softmax: scores are bounded), so S^T
    already includes the ALiBi bias.
  - Causality: k-tiles fully above the diagonal are skipped; diagonal
    tiles are zero-filled post-exp with gpsimd.affine_select.
  - V carries a ones column per head, so the PV matmul yields O^T plus the
    softmax denominators; O^T *= 1/den via DVE reciprocal + PE broadcast.
"""
import math

import ml_dtypes
import numpy as np

BF = ml_dtypes.bfloat16

import concourse.bass as bass
import concourse.tile as tile
from concourse import mybir, bacc
from concourse.bass_utils import run_bass_kernel_spmd

F32 = mybir.dt.float32
F32R = mybir.dt.float32r
BF16 = mybir.dt.bfloat16

B, T, C, H = 2, 2048, 1024, 16
D = C // H            # 64 head dim
NCORES = 8
HG = 4                # heads per core
CG = HG * D           # 256 channels per core
VW = HG * (D + 1)     # 260: V with a ones column per head
QB = 512              # q block width
KTW = 128             # k tile width
NQB = T // QB         # 4
NKT = T // KTW        # 16
NCH = C // 128        # 8 contraction chunks


def _slopes(n):
    def p2(m):
        start = 2 ** (-(2 ** -(math.log2(m) - 3)))
        return [start * start**i for i in range(m)]
    if math.log2(n).is_integer():
        return p2(n)
    c = 2 ** math.floor(math.log2(n))
    return p2(c) + _slopes(2 * c)[0::2][: n - c]


def _build():
    nc = bacc.Bacc()
    xt = nc.declare_dram_parameter("xt", [C + 1, T], BF16, isOutput=False)
    wq = nc.declare_dram_parameter("wq", [C + 1, CG], BF16, isOutput=False)
    wk = nc.declare_dram_parameter("wk", [C + 1, CG], BF16, isOutput=False)
    wv = nc.declare_dram_parameter("wv", [C + 1, VW], BF16, isOutput=False)
    wo = nc.declare_dram_parameter("wo", [CG, C], BF16, isOutput=False)
    hka = nc.declare_dram_parameter("hka", [HG, 2, T], F32R, isOutput=False)
    hqa = nc.declare_dram_parameter("hqa", [HG, 2, T], F32R, isOutput=False)
    stair = nc.declare_dram_parameter("stair", [128, 640], F32R, isOutput=False)
    ident = nc.declare_dram_parameter("ident", [128, 128], F32R, isOutput=False)
    hbias = nc.declare_dram_parameter("hbias", [128, 128], F32, isOutput=False)
    y = nc.declare_dram_parameter("y", [T, C], BF16, isOutput=True)

    EXP = mybir.ActivationFunctionType.Exp
    CPY = mybir.ActivationFunctionType.Copy

    with tile.TileContext(nc) as tc, \
         nc.allow_low_precision(reason="fp32r compute"):
        with tc.tile_pool(name="const", bufs=1) as cp, \
             tc.tile_pool(name="xtp", bufs=20) as xtp, \
             tc.tile_pool(name="qap", bufs=8) as qap, \
             tc.tile_pool(name="otp", bufs=4) as otp, \
             tc.tile_pool(name="ptp", bufs=6) as ptp, \
             tc.tile_pool(name="yp", bufs=2) as ypool, \
             tc.tile_pool(name="misc", bufs=2) as mp, \
             tc.tile_pool(name="ps", bufs=6, space="PSUM") as psp, \
             tc.tile_pool(name="po", bufs=2, space="PSUM") as pop:

            # ---- constants: weights, aug rows, zero fill ----
            # DMA emission order matters for time-to-first-matmul: wq and
            # the first x block go first so the Q projection can start while
            # the rest of the constants stream in.
            wq_sb = [cp.tile([128, CG], BF16, tag=f"wq{c}", name=f"wq{c}") for c in range(NCH)]
            wk_sb = [cp.tile([128, CG], BF16, tag=f"wk{c}", name=f"wk{c}") for c in range(NCH)]
            wv_sb = [cp.tile([128, VW], BF16, tag=f"wv{c}", name=f"wv{c}") for c in range(NCH)]
            wo_sb = [cp.tile([128, C], BF16, tag=f"wo{c}", name=f"wo{c}") for c in range(2)]
            wqb = cp.tile([1, CG], BF16, tag="wqb")
            wkb = cp.tile([1, CG], BF16, tag="wkb")
            wvb = cp.tile([1, VW], BF16, tag="wvb")
            ones_sb = cp.tile([1, QB], BF16, tag="ones")
            ones_fr = cp.tile([1, 128], F32R, tag="ones_fr")
            ones32 = cp.tile([1, 128], F32, tag="ones32")
            nc.vector.memset(ones32[:], 1.0)
            nc.vector.tensor_copy(ones_fr[:], ones32[:])
            xts0 = []
            for c in range(NCH):
                nc.sync.dma_start(wq_sb[c][:], wq[128 * c:128 * (c + 1), :])
                xtt = xtp.tile([128, QB], BF16, tag="xt", name=f"xt0_{c}")
                nc.sync.dma_start(xtt[:], xt[128 * c:128 * (c + 1), 0:QB])
                xts0.append(xtt)
            nc.sync.dma_start(wqb[:], wq[C:C + 1, :])
            nc.sync.dma_start(ones_sb[:], xt[C:C + 1, 0:QB])

            for c in range(NCH):
                nc.sync.dma_start(wk_sb[c][:], wk[128 * c:128 * (c + 1), :])
            nc.sync.dma_start(wkb[:], wk[C:C + 1, :])

            zf = cp.tile([128, QB], F32, tag="zf")
            nc.vector.memset(zf[:], 0.0)

            # causal-mask staircase: stair[p, f] = -3000 where f - 128 < p.
            # Accumulating I.T @ stair[:, off:off+W] into a diagonal S tile
            # drives masked (k > q) scores to -3000 pre-exp, so the exp
            # underflows to 0 and no post-exp select is needed.  These (and
            # the ka aug rows) are needed before wv/wo, so they DMA first.
            stair_sb = cp.tile([128, 640], F32R, tag="stair")
            ident_sb = cp.tile([128, 128], F32R, tag="ident")
            hb_sb = cp.tile([128, 128], F32, tag="hb")
            nc.sync.dma_start(stair_sb[:], stair[:])
            nc.sync.dma_start(ident_sb[:], ident[:])
            nc.sync.dma_start(hb_sb[:], hbias[:])

            # Slots 0,1 (steep ALiBi slopes): per-head K^T tiles with the
            # rank-2 aug-row ALiBi.  Even head: data rows 0:64, aug rows
            # 64:66, zeros 66:128.  Odd head: aug 0:2, zeros 2:64, data
            # 64:128.  K aug = (slope8*k, ones).
            # Slots 2,3 (shallow slopes): one packed [128, T] K^T tile, slot2
            # on rows 0:64 and slot3 on rows 64:128; their ALiBi rides the
            # exp as a per-partition ACT bias slope*(k - q0) (the per-q part
            # cancels between softmax numerator and denominator), so the two
            # slots' S matmuls row-tile the PE concurrently.
            kap23 = cp.tile([128, T], BF16, tag="kap23")
            ka = [cp.tile([128, T], F32R, tag=f"ka{h}", name=f"ka{h}") for h in range(2)]
            for h in range(2):
                par = h % 2
                arow = 64 if par == 0 else 0
                # zero the whole non-data half (32-aligned partition base),
                # then the aug-row DMA overwrites its 2 rows
                for blk in range(NQB):
                    sl = slice(QB * blk, QB * (blk + 1))
                    nc.vector.tensor_copy(ka[h][arow:arow + 64, sl],
                                          zf[arow:arow + 64, :])
                nc.sync.dma_start(ka[h][arow:arow + 2, :], hka[h])

            for c in range(NCH):
                nc.sync.dma_start(wv_sb[c][:], wv[128 * c:128 * (c + 1), :])
            nc.sync.dma_start(wvb[:], wv[C:C + 1, :])
            for c in range(2):
                nc.sync.dma_start(wo_sb[c][:], wo[128 * c:128 * (c + 1), :])

            v_sb = [cp.tile([128, VW], F32R, tag=f"v{t}", name=f"v{t}") for t in range(NKT)]

            # ---- fused, software-pipelined per-block loop ----
            def proj(qb):
                """QKV projections for t-block qb; returns the Q tiles."""
                tsl = slice(QB * qb, QB * (qb + 1))
                if qb == 0:
                    xts = xts0
                else:
                    xts = []
                    for c in range(NCH):
                        xtt = xtp.tile([128, QB], BF16, tag="xt",
                                       name=f"xt{qb}_{c}")
                        nc.sync.dma_start(xtt[:],
                                          xt[128 * c:128 * (c + 1), tsl])
                        xts.append(xtt)

                qa_t = []
                for h in range(2):
                    qat = qap.tile([128, QB], F32R, tag="qa",
                                   name=f"qa{qb}_{h}")
                    par = h % 2
                    arow = 64 if par == 0 else 0
                    nc.vector.tensor_copy(qat[arow:arow + 64, :],
                                          zf[arow:arow + 64, :])
                    nc.sync.dma_start(qat[arow:arow + 2, :], hqa[h][:, tsl])
                    qa_t.append(qat)
                q23 = qap.tile([128, QB], BF16, tag="q23",
                               name=f"q23_{qb}")
                qa_t.append(q23)

                for wsb, wb, is_q in ((wq_sb, wqb, True), (wk_sb, wkb, False)):
                    for m in range(2):
                        ps = psp.tile([128, QB], F32, tag="ps")
                        for c in range(NCH):
                            nc.tensor.matmul(
                                ps[:], wsb[c][:, 128 * m:128 * (m + 1)],
                                xts[c][:], start=(c == 0), stop=False,
                                skip_group_check=True)
                        nc.tensor.matmul(
                            ps[:], wb[:, 128 * m:128 * (m + 1)], ones_sb[:],
                            start=False, stop=True, skip_group_check=True)
                        if m == 1:
                            # packed pair: slot2 rows 0:64, slot3 rows
                            # 64:128, exactly the proj PSUM layout
                            if is_q:
                                nc.vector.tensor_copy(q23[:], ps[:])
                            else:
                                nc.vector.tensor_copy(kap23[:, tsl], ps[:])
                            continue
                        for j in range(2):
                            h = 2 * m + j
                            rows = slice(64 * j, 64 * j + 64)
                            if is_q:
                                nc.vector.tensor_copy(qa_t[h][rows, :],
                                                      ps[rows, :])
                            else:
                                nc.vector.tensor_copy(ka[h][rows, tsl],
                                                      ps[rows, :])

                for tt in range(4):
                    kt = 4 * qb + tt
                    psv = psp.tile([128, QB], F32, tag="ps")
                    for c in range(NCH):
                        nc.tensor.matmul(
                            psv[:, 0:VW],
                            xts[c][:, 128 * tt:128 * (tt + 1)], wv_sb[c][:],
                            start=(c == 0), stop=False, skip_group_check=True)
                    nc.tensor.matmul(
                        psv[:, 0:VW], ones_sb[:, 0:128], wvb[:],
                        start=False, stop=True, skip_group_check=True)
                    nc.vector.tensor_copy(v_sb[kt][:], psv[:, 0:VW])
                return qa_t

            qa_next = proj(0)
            for qb in range(NQB):
                qa_t = qa_next
                # attention for this q-block.  Pass A per head is the
                # PE-heavy S/exp/mask/PV chain; pass B (recip -> broadcast
                # -> divide) for head h is emitted after head h+1's pass A
                # so the broadcast matmul never sits at the front of the PE
                # queue waiting on the DVE reciprocal.
                po_t = {}
                ot_t = [otp.tile([128, QB], BF16, tag="ot",
                                 name=f"ot_{qb}_{c}") for c in range(2)]

                # ALiBi windows per head slot: with the strided head
                # assignment, slot j holds global heads {4j..4j+3}; a tile
                # whose every (k, q) pair has slope*(k-q) <= -14 contributes
                # < 1e-4 relative attention mass (well under the 2e-2 rel-err
                # budget).  W_j = 14 / min-slope-in-slot.
                WIN = (56.0, 224.0, 897.0, 1e9)

                # Diagonal k-tile tt (tt = kt - 4*qb) only matters for q
                # columns >= 128*tt, so trim its S/exp/PV to [C_tt, 512).
                # tt=3 keeps 256 cols (f32r needs a >=256 moving dim); its
                # extra cols [256,384) are fully masked by the staircase.
                TRIM = ((0, QB), (128, 384), (256, 256), (256, 256))

                def finish_head(h, po):
                    den = mp.tile([1, QB], F32, tag="den", bufs=2,
                                  name=f"den_{qb}_{h}")
                    nc.vector.tensor_copy(den[:], po[D:D + 1, :])
                    rc32 = mp.tile([1, QB], F32, tag="rc32", bufs=2,
                                   name=f"rc32_{qb}_{h}")
                    nc.vector.reciprocal_approx_fast(rc32[:], den[:])
                    rc = mp.tile([1, QB], F32R, tag="rc", bufs=4,
                                 name=f"rc_{qb}_{h}")
                    nc.vector.tensor_copy(rc[:], rc32[:])
                    po_t[h] = (po, rc)

                def pass_a(h):
                    # diagonal tiles go first so tile tt=0 opens the full
                    # [0,512) PV accumulation region and the head's tail is
                    # short-latency
                    full = [kt for kt in range(4 * qb)
                            if 128 * kt > QB * qb - WIN[h] - 127]
                    kts = list(range(4 * qb, 4 * qb + 4)) + full
                    po = pop.tile([D + 1, QB], F32, tag="po",
                                  name=f"po_{qb}_{h}")
                    for i, kt in enumerate(kts):
                        tt = kt - 4 * qb
                        if tt >= 0:
                            c0, w = TRIM[tt]
                        else:
                            c0, w = 0, QB
                        pss = psp.tile([128, QB], F32, tag="ps")
                        nc.tensor.matmul(
                            pss[:, 0:w], ka[h][:, 128 * kt:128 * (kt + 1)],
                            qa_t[h][:, c0:c0 + w], start=True, stop=(tt < 0),
                            skip_group_check=True)
                        if tt >= 0:
                            # masked (k > q) entries get -3000 pre-exp
                            soff = 0 if tt == 3 else 128
                            nc.tensor.matmul(
                                pss[:, 0:w], ident_sb[:],
                                stair_sb[:, soff:soff + w], start=False,
                                stop=True, skip_group_check=True)
                        pt = ptp.tile([128, QB], F32R, tag="pt")
                        nc.scalar.activation(pt[:, 0:w], pss[:, 0:w], EXP,
                                             bias=0.0, scale=0.125)
                        nc.tensor.matmul(
                            po[:, c0:c0 + w],
                            v_sb[kt][:, 65 * h:65 * (h + 1)], pt[:, 0:w],
                            start=(i == 0), stop=(i == len(kts) - 1),
                            skip_group_check=True)
                    finish_head(h, po)

                def pass_a23():
                    # slots 2,3 share one packed K/Q tile; common k-tiles
                    # issue as two concurrent row-tiled S matmuls
                    full2 = [kt for kt in range(4 * qb)
                             if 128 * kt > QB * qb - WIN[2] - 127]
                    kts = list(range(4 * qb, 4 * qb + 4)) + list(range(4 * qb))
                    po2 = pop.tile([D + 1, QB], F32, tag="po",
                                   name=f"po_{qb}_2")
                    po3 = pop.tile([D + 1, QB], F32, tag="po",
                                   name=f"po_{qb}_3")
                    n2 = 4 + len(full2)
                    n3 = len(kts)
                    i2 = i3 = 0
                    for kt in kts:
                        tt = kt - 4 * qb
                        if tt >= 0:
                            c0, w = TRIM[tt]
                        else:
                            c0, w = 0, QB
                        ktsl = slice(128 * kt, 128 * (kt + 1))
                        has2 = tt >= 0 or kt in full2
                        if has2:
                            pss2 = psp.tile([128, QB], F32, tag="ps")
                            nc.tensor.matmul(
                                pss2[:, 0:w], kap23[0:64, ktsl],
                                qa_t[2][0:64, c0:c0 + w], start=True,
                                stop=(tt < 0), skip_group_check=True,
                                tile_position=(0, 0))
                        pss3 = psp.tile([128, QB], F32, tag="ps")
                        nc.tensor.matmul(
                            pss3[:, 0:w], kap23[64:128, ktsl],
                            qa_t[2][64:128, c0:c0 + w], start=True,
                            stop=(tt < 0), skip_group_check=True,
                            tile_position=(64, 0))
                        if tt >= 0:
                            soff = 0 if tt == 3 else 128
                            nc.tensor.matmul(
                                pss2[:, 0:w], ident_sb[:],
                                stair_sb[:, soff:soff + w], start=False,
                                stop=True, skip_group_check=True)
                            nc.tensor.matmul(
                                pss3[:, 0:w], ident_sb[:],
                                stair_sb[:, soff:soff + w], start=False,
                                stop=True, skip_group_check=True)
                        if has2:
                            bcol = 16 * qb + kt
                            pt2 = ptp.tile([128, QB], F32R, tag="pt")
                            nc.scalar.activation(
                                pt2[:, 0:w], pss2[:, 0:w], EXP,
                                bias=hb_sb[:, bcol:bcol + 1], scale=0.125)
                            nc.tensor.matmul(
                                po2[:, c0:c0 + w],
                                v_sb[kt][:, 65 * 2:65 * 3], pt2[:, 0:w],
                                start=(i2 == 0), stop=(i2 == n2 - 1),
                                skip_group_check=True)
                            i2 += 1
                        bcol = 64 + 16 * qb + kt
                        pt3 = ptp.tile([128, QB], F32R, tag="pt")
                        nc.scalar.activation(
                            pt3[:, 0:w], pss3[:, 0:w], EXP,
                            bias=hb_sb[:, bcol:bcol + 1], scale=0.125)
                        nc.tensor.matmul(
                            po3[:, c0:c0 + w],
                            v_sb[kt][:, 65 * 3:65 * 4], pt3[:, 0:w],
                            start=(i3 == 0), stop=(i3 == n3 - 1),
                            skip_group_check=True)
                        i3 += 1
                    finish_head(3, po3)
                    finish_head(2, po2)

                def pass_b(h):
                    po, rc = po_t.pop(h)
                    pb = psp.tile([D, QB], F32, tag="ps",
                                  name=f"pb_{qb}_{h}")
                    nc.tensor.matmul(pb[:], ones_fr[:, 0:D], rc[:],
                                     start=True, stop=True,
                                     skip_group_check=True)
                    bc = mp.tile([D, QB], F32, tag="bc", bufs=4,
                                 name=f"bc_{qb}_{h}")
                    nc.vector.tensor_copy(bc[:], pb[:])
                    pair = ot_t[h // 2]
                    if h % 2 == 0:
                        nc.vector.tensor_tensor(pair[0:D, :], po[0:D, :],
                                                bc[:],
                                                op=mybir.AluOpType.mult)
                    else:
                        # odd head's O^T lands at partitions 0:64; DVE
                        # cannot shift partitions, so divide into a temp
                        # then DMA it into rows 64:128 of the pair tile
                        tmp = mp.tile([D, QB], BF16, tag="ottmp", bufs=4,
                                      name=f"ottmp_{qb}_{h}")
                        nc.vector.tensor_tensor(tmp[:], po[0:D, :], bc[:],
                                                op=mybir.AluOpType.mult)
                        # scalar HWDGE queue: keeps the Sync queue (which
                        # carries the xt prefetch) free of this hop
                        nc.scalar.dma_start(pair[D:2 * D, :], tmp[:])

                # Slots 2,3 (packed pair) first, then 1, then 0, so the qb's
                # trailing pass_b chain ends on even head 0 (no DMA hop); the
                # output projection starts on pair 1 (ready mid-sequence).
                pass_a23()
                pass_a(1)
                pass_b(3)
                pass_a(0)
                pass_b(2)

                # next q-block's projections are emitted BEFORE the last two
                # pass_b's and the output projection: the PE queue is
                # in-order, so these ready proj matmuls cover the ~3us DVE
                # recip/bcast chains of the trailing heads.
                if qb + 1 < NQB:
                    qa_next = proj(qb + 1)
                pass_b(1)
                pass_b(0)

                # output projection for this t-block (pair 1 first)
                for tt in range(4):
                    t = 4 * qb + tt
                    fsl = slice(128 * tt, 128 * (tt + 1))
                    ysb = ypool.tile([128, C], BF16, tag="y",
                                     name=f"y_{qb}_{tt}")
                    for half in range(2):
                        hsl = slice(QB * half, QB * (half + 1))
                        py = psp.tile([128, QB], F32, tag="ps")
                        for c in (1, 0):
                            nc.tensor.matmul(
                                py[:], ot_t[c][:, fsl], wo_sb[c][:, hsl],
                                start=(c == 1), stop=(c == 0),
                                skip_group_check=True)
                        nc.scalar.activation(ysb[:, hsl], py[:], CPY)
                        # sync queue: emitted after proj(qb+1)'s prefetch
                        # triggers, so these can't block the next q-block
                        nc.sync.dma_start(y[128 * t:128 * (t + 1), hsl],
                                          ysb[:, hsl])
    nc.finalize()
    return nc


_NC_CACHE = None


def _get_nc():
    global _NC_CACHE
    if _NC_CACHE is None:
        _NC_CACHE = _build()
    return _NC_CACHE


def kernel(x, Wq, bq, Wk, bk, Wv, bv, Wo, bo):
    x = np.asarray(x, dtype=np.float32)
    Wq, bq = np.asarray(Wq, np.float32), np.asarray(bq, np.float32)
    Wk, bk = np.asarray(Wk, np.float32), np.asarray(bk, np.float32)
    Wv, bv = np.asarray(Wv, np.float32), np.asarray(bv, np.float32)
    Wo, bo = np.asarray(Wo, np.float32), np.asarray(bo, np.float32)

    slopes = np.asarray(_slopes(H), dtype=np.float32)
    ar = np.arange(T, dtype=np.float32)

    pp, ff = np.meshgrid(np.arange(128), np.arange(640), indexing="ij")
    stair_np = np.where(ff - 128 < pp, -3000.0, 0.0).astype(np.float32)
    ident_np = np.eye(128, dtype=np.float32)

    xts = []
    for b in range(B):
        xa = np.empty((C + 1, T), np.float32)
        xa[:C] = x[b].T
        xa[C] = 1.0
        xts.append(np.ascontiguousarray(xa.astype(BF)))

    pr = np.arange(128, dtype=np.float32)
    shards = []
    for g in range(HG):
        # strided head assignment: core g, slot j <-> global head 4j+g, so
        # each slot's ALiBi slope range is uniform across cores and the
        # (SPMD-shared) graph can window steep slots' attention
        heads = [HG * j + g for j in range(HG)]
        # ACT-bias table for slots 2,3: col = 64*(slot-2) + 16*qb + kt,
        # value[p] = slope * (128*kt + p - 512*qb)
        hb = np.zeros((128, 128), np.float32)
        for sl in (2, 3):
            s = slopes[heads[sl]]
            for qbn in range(4):
                for kt in range(16):
                    col = 64 * (sl - 2) + 16 * qbn + kt
                    hb[:, col] = s * (128.0 * kt + pr - 512.0 * qbn)
        cols = np.concatenate([np.arange(D * h, D * (h + 1)) for h in heads])
        wqa = np.concatenate([Wq[:, cols], bq[None, cols]], axis=0)
        wka = np.concatenate([Wk[:, cols], bk[None, cols]], axis=0)
        wva = np.zeros((C + 1, VW), np.float32)
        for j, h in enumerate(heads):
            hsl = slice(D * h, D * (h + 1))
            wva[:C, 65 * j:65 * j + D] = Wv[:, hsl]
            wva[C, 65 * j:65 * j + D] = bv[hsl]
            wva[C, 65 * j + D] = 1.0
        woa = np.ascontiguousarray(Wo[cols, :])
        hk = np.empty((HG, 2, T), np.float32)
        hq = np.empty((HG, 2, T), np.float32)
        for j, h in enumerate(heads):
            # K rows (k, s8) pair with Q rows (s8, -q): S += s8*(k - q).
            # Integer k/q are exact on the f32r grid and s8 rounds once, so
            # the large terms cancel exactly in the fp32 PSUM accumulator
            # (splitting s8*k / s8*q would round each entry independently
            # and leave O(s8*T*eps) noise in the scores).
            s8 = 8.0 * slopes[h]
            hk[j, 0] = ar
            hk[j, 1] = s8
            hq[j, 0] = s8
            hq[j, 1] = -ar
        shards.append(dict(
            wq=np.ascontiguousarray(wqa.astype(BF)),
            wk=np.ascontiguousarray(wka.astype(BF)),
            wv=wva.astype(BF), wo=np.ascontiguousarray(woa.astype(BF)),
            hka=hk, hqa=hq, stair=stair_np, ident=ident_np, hbias=hb))

    in_maps = []
    for core in range(NCORES):
        b, g = divmod(core, HG)
        m = dict(shards[g])
        m["xt"] = xts[b]
        in_maps.append(m)

    nc = _get_nc()
    res = run_bass_kernel_spmd(nc, in_maps, core_ids=list(range(NCORES)))

    out = np.empty((B, T, C), np.float32)
    for b in range(B):
        acc = res.results[4 * b]["y"].astype(np.float32).copy()
        for g in range(1, HG):
            acc += res.results[4 * b + g]["y"].astype(np.float32)
        out[b] = acc + bo[None, :]
    return out



# revision 47
# speedup vs baseline: 1.1328x; 1.0512x over previous
"""ALiBi causal attention layer on 8 TRN2 NeuronCores.

Sharding: data parallel on batch (B=2) x tensor parallel on heads (16 -> 4
groups of 4).  Core c = 4*b + g computes, for batch element b, the STRIDED
head set {g, 4+g, 8+g, 12+g} end to end: QKV projections (column-sharded),
causal ALiBi attention, and the row-sharded output projection.  The host
sums the 4 partial outputs per batch element (the tensor-parallel
all-reduce) and adds the output bias.  The striding makes head slot j hold
global heads {4j..4j+3} on every core, so each slot's ALiBi slope range is
uniform and the SPMD-shared graph can window steep slots' attention: slot 0
(slopes >= 0.25) looks back only 120 positions, slot 1 (>= 0.0625) 480 --
skipped k-tiles contribute < 1e-11 to the softmax.

Device kernel (all matmuls in float32r, ~1e-4 rel err, fp32 PSUM accum):
  - x arrives host-transposed with a ones row: xt [1025, 2048]; projection
    biases ride in an augmented contraction row of each weight matrix.
  - K^T lives in per-head [128, 2048] tiles: head data at its native
    partition parity (even head rows 0:64, odd rows 64:128), the ALiBi
    rank-2 rows (slope*8*k, ones) adjacent, remaining rows zeroed.  Q^T
    uses matching per-(head, q-block) [128, 512] tiles with rows
    (ones, -slope*8*q).  S^T = K_aug^T.T @ Q_aug then exp() directly on
    ACT with scale=1/8 (max-free softmax: scores are bounded), so S^T
    already includes the ALiBi bias.
  - Causality: k-tiles fully above the diagonal are skipped; diagonal
    tiles are zero-filled post-exp with gpsimd.affine_select.
  - V carries a ones column per head, so the PV matmul yields O^T plus the
    softmax denominators; O^T *= 1/den via DVE reciprocal + PE broadcast.
"""
import math

import ml_dtypes
import numpy as np

BF = ml_dtypes.bfloat16

import concourse.bass as bass
import concourse.tile as tile
from concourse import mybir, bacc
from concourse.bass_utils import run_bass_kernel_spmd

F32 = mybir.dt.float32
F32R = mybir.dt.float32r
BF16 = mybir.dt.bfloat16

B, T, C, H = 2, 2048, 1024, 16
D = C // H            # 64 head dim
NCORES = 8
HG = 4                # heads per core
CG = HG * D           # 256 channels per core
VW = HG * (D + 1)     # 260: V with a ones column per head
QB = 512              # q block width
KTW = 128             # k tile width
NQB = T // QB         # 4
NKT = T // KTW        # 16
NCH = C // 128        # 8 contraction chunks


def _slopes(n):
    def p2(m):
        start = 2 ** (-(2 ** -(math.log2(m) - 3)))
        return [start * start**i for i in range(m)]
    if math.log2(n).is_integer():
        return p2(n)
    c = 2 ** math.floor(math.log2(n))
    return p2(c) + _slopes(2 * c)[0::2][: n - c]


def _build():
    nc = bacc.Bacc()
    xt = nc.declare_dram_parameter("xt", [C + 1, T], BF16, isOutput=False)
    wq = nc.declare_dram_parameter("wq", [C + 1, CG], BF16, isOutput=False)
    wk = nc.declare_dram_parameter("wk", [C + 1, CG], BF16, isOutput=False)
    wv = nc.declare_dram_parameter("wv", [C + 1, VW], BF16, isOutput=False)
    wo = nc.declare_dram_parameter("wo", [CG, C], BF16, isOutput=False)
    hka = nc.declare_dram_parameter("hka", [HG, 2, T], F32R, isOutput=False)
    hqa = nc.declare_dram_parameter("hqa", [HG, 2, T], F32R, isOutput=False)
    stair = nc.declare_dram_parameter("stair", [128, 640], BF16, isOutput=False)
    ident = nc.declare_dram_parameter("ident", [128, 128], BF16, isOutput=False)
    hbias = nc.declare_dram_parameter("hbias", [128, 128], F32, isOutput=False)
    y = nc.declare_dram_parameter("y", [T, C], BF16, isOutput=True)

    EXP = mybir.ActivationFunctionType.Exp
    CPY = mybir.ActivationFunctionType.Copy

    with tile.TileContext(nc) as tc, \
         nc.allow_low_precision(reason="fp32r compute"):
        with tc.tile_pool(name="const", bufs=1) as cp, \
             tc.tile_pool(name="xtp", bufs=20) as xtp, \
             tc.tile_pool(name="qap", bufs=8) as qap, \
             tc.tile_pool(name="otp", bufs=4) as otp, \
             tc.tile_pool(name="ptp", bufs=6) as ptp, \
             tc.tile_pool(name="yp", bufs=2) as ypool, \
             tc.tile_pool(name="misc", bufs=2) as mp, \
             tc.tile_pool(name="ps", bufs=6, space="PSUM") as psp, \
             tc.tile_pool(name="po", bufs=2, space="PSUM") as pop:

            # ---- constants: weights, aug rows, zero fill ----
            # DMA emission order matters for time-to-first-matmul: wq and
            # the first x block go first so the Q projection can start while
            # the rest of the constants stream in.
            wq_sb = [cp.tile([128, CG], BF16, tag=f"wq{c}", name=f"wq{c}") for c in range(NCH)]
            wk_sb = [cp.tile([128, CG], BF16, tag=f"wk{c}", name=f"wk{c}") for c in range(NCH)]
            wv_sb = [cp.tile([128, VW], BF16, tag=f"wv{c}", name=f"wv{c}") for c in range(NCH)]
            wo_sb = [cp.tile([128, C], BF16, tag=f"wo{c}", name=f"wo{c}") for c in range(2)]
            wqb = cp.tile([1, CG], BF16, tag="wqb")
            wkb = cp.tile([1, CG], BF16, tag="wkb")
            wvb = cp.tile([1, VW], BF16, tag="wvb")
            ones_sb = cp.tile([1, QB], BF16, tag="ones")
            ones_fr = cp.tile([1, 128], F32R, tag="ones_fr")
            ones32 = cp.tile([1, 128], F32, tag="ones32")
            nc.vector.memset(ones32[:], 1.0)
            nc.vector.tensor_copy(ones_fr[:], ones32[:])
            xts0 = []
            for c in range(NCH):
                nc.sync.dma_start(wq_sb[c][:], wq[128 * c:128 * (c + 1), :])
                xtt = xtp.tile([128, QB], BF16, tag="xt", name=f"xt0_{c}")
                nc.sync.dma_start(xtt[:], xt[128 * c:128 * (c + 1), 0:QB])
                xts0.append(xtt)
            nc.sync.dma_start(wqb[:], wq[C:C + 1, :])
            nc.sync.dma_start(ones_sb[:], xt[C:C + 1, 0:QB])

            for c in range(NCH):
                nc.sync.dma_start(wk_sb[c][:], wk[128 * c:128 * (c + 1), :])
            nc.sync.dma_start(wkb[:], wk[C:C + 1, :])

            zf = cp.tile([128, QB], F32, tag="zf")
            nc.vector.memset(zf[:], 0.0)

            # causal-mask staircase: stair[p, f] = -3000 where f - 128 < p.
            # Accumulating I.T @ stair[:, off:off+W] into a diagonal S tile
            # drives masked (k > q) scores to -3000 pre-exp, so the exp
            # underflows to 0 and no post-exp select is needed.  These (and
            # the ka aug rows) are needed before wv/wo, so they DMA first.
            stair_sb = cp.tile([128, 640], BF16, tag="stair")
            ident_sb = cp.tile([128, 128], BF16, tag="ident")
            hb_sb = cp.tile([128, 128], F32, tag="hb")
            nc.sync.dma_start(stair_sb[:], stair[:])
            nc.sync.dma_start(ident_sb[:], ident[:])
            nc.sync.dma_start(hb_sb[:], hbias[:])

            # Slots 0,1 (steep ALiBi slopes): per-head K^T tiles with the
            # rank-2 aug-row ALiBi.  Even head: data rows 0:64, aug rows
            # 64:66, zeros 66:128.  Odd head: aug 0:2, zeros 2:64, data
            # 64:128.  K aug = (slope8*k, ones).
            # Slots 2,3 (shallow slopes): one packed [128, T] K^T tile, slot2
            # on rows 0:64 and slot3 on rows 64:128; their ALiBi rides the
            # exp as a per-partition ACT bias slope*(k - q0) (the per-q part
            # cancels between softmax numerator and denominator), so the two
            # slots' S matmuls row-tile the PE concurrently.
            kap23 = cp.tile([128, T], BF16, tag="kap23")
            ka = [cp.tile([128, T], F32R, tag=f"ka{h}", name=f"ka{h}") for h in range(2)]
            for h in range(2):
                par = h % 2
                arow = 64 if par == 0 else 0
                # zero the whole non-data half (32-aligned partition base),
                # then the aug-row DMA overwrites its 2 rows
                for blk in range(NQB):
                    sl = slice(QB * blk, QB * (blk + 1))
                    nc.vector.tensor_copy(ka[h][arow:arow + 64, sl],
                                          zf[arow:arow + 64, :])
                nc.sync.dma_start(ka[h][arow:arow + 2, :], hka[h])

            for c in range(NCH):
                nc.sync.dma_start(wv_sb[c][:], wv[128 * c:128 * (c + 1), :])
            nc.sync.dma_start(wvb[:], wv[C:C + 1, :])
            for c in range(2):
                nc.sync.dma_start(wo_sb[c][:], wo[128 * c:128 * (c + 1), :])

            v_sb = [cp.tile([128, VW], F32R, tag=f"v{t}", name=f"v{t}") for t in range(NKT)]

            # ---- fused, software-pipelined per-block loop ----
            def proj(qb):
                """QKV projections for t-block qb; returns the Q tiles."""
                tsl = slice(QB * qb, QB * (qb + 1))
                if qb == 0:
                    xts = xts0
                else:
                    xts = []
                    for c in range(NCH):
                        xtt = xtp.tile([128, QB], BF16, tag="xt",
                                       name=f"xt{qb}_{c}")
                        nc.sync.dma_start(xtt[:],
                                          xt[128 * c:128 * (c + 1), tsl])
                        xts.append(xtt)

                qa_t = []
                for h in range(2):
                    qat = qap.tile([128, QB], F32R, tag="qa",
                                   name=f"qa{qb}_{h}")
                    par = h % 2
                    arow = 64 if par == 0 else 0
                    nc.vector.tensor_copy(qat[arow:arow + 64, :],
                                          zf[arow:arow + 64, :])
                    nc.sync.dma_start(qat[arow:arow + 2, :], hqa[h][:, tsl])
                    qa_t.append(qat)
                q23 = qap.tile([128, QB], BF16, tag="q23",
                               name=f"q23_{qb}")
                qa_t.append(q23)

                for wsb, wb, is_q in ((wq_sb, wqb, True), (wk_sb, wkb, False)):
                    for m in range(2):
                        ps = psp.tile([128, QB], F32, tag="ps")
                        for c in range(NCH):
                            nc.tensor.matmul(
                                ps[:], wsb[c][:, 128 * m:128 * (m + 1)],
                                xts[c][:], start=(c == 0), stop=False,
                                skip_group_check=True)
                        nc.tensor.matmul(
                            ps[:], wb[:, 128 * m:128 * (m + 1)], ones_sb[:],
                            start=False, stop=True, skip_group_check=True)
                        if m == 1:
                            # packed pair: slot2 rows 0:64, slot3 rows
                            # 64:128, exactly the proj PSUM layout
                            if is_q:
                                nc.vector.tensor_copy(q23[:], ps[:])
                            else:
                                nc.vector.tensor_copy(kap23[:, tsl], ps[:])
                            continue
                        for j in range(2):
                            h = 2 * m + j
                            rows = slice(64 * j, 64 * j + 64)
                            if is_q:
                                nc.vector.tensor_copy(qa_t[h][rows, :],
                                                      ps[rows, :])
                            else:
                                nc.vector.tensor_copy(ka[h][rows, tsl],
                                                      ps[rows, :])

                for tt in range(4):
                    kt = 4 * qb + tt
                    psv = psp.tile([128, QB], F32, tag="ps")
                    for c in range(NCH):
                        nc.tensor.matmul(
                            psv[:, 0:VW],
                            xts[c][:, 128 * tt:128 * (tt + 1)], wv_sb[c][:],
                            start=(c == 0), stop=False, skip_group_check=True)
                    nc.tensor.matmul(
                        psv[:, 0:VW], ones_sb[:, 0:128], wvb[:],
                        start=False, stop=True, skip_group_check=True)
                    nc.vector.tensor_copy(v_sb[kt][:], psv[:, 0:VW])
                return qa_t

            qa_next = proj(0)
            for qb in range(NQB):
                qa_t = qa_next
                # attention for this q-block.  Pass A per head is the
                # PE-heavy S/exp/mask/PV chain; pass B (recip -> broadcast
                # -> divide) for head h is emitted after head h+1's pass A
                # so the broadcast matmul never sits at the front of the PE
                # queue waiting on the DVE reciprocal.
                po_t = {}
                ot_t = [otp.tile([128, QB], BF16, tag="ot",
                                 name=f"ot_{qb}_{c}") for c in range(2)]

                # ALiBi windows per head slot: with the strided head
                # assignment, slot j holds global heads {4j..4j+3}; a tile
                # whose every (k, q) pair has slope*(k-q) <= -14 contributes
                # < 1e-4 relative attention mass (well under the 2e-2 rel-err
                # budget).  W_j = 14 / min-slope-in-slot.
                WIN = (56.0, 224.0, 897.0, 1e9)

                # Diagonal k-tile tt (tt = kt - 4*qb) only matters for q
                # columns >= 128*tt, so trim its S/exp/PV to [C_tt, 512).
                # tt=3 keeps 256 cols (f32r needs a >=256 moving dim); its
                # extra cols [256,384) are fully masked by the staircase.
                TRIM = ((0, QB), (128, 384), (256, 256), (256, 256))

                def finish_head(h, po):
                    den = mp.tile([1, QB], F32, tag="den", bufs=2,
                                  name=f"den_{qb}_{h}")
                    nc.vector.tensor_copy(den[:], po[D:D + 1, :])
                    rc32 = mp.tile([1, QB], F32, tag="rc32", bufs=2,
                                   name=f"rc32_{qb}_{h}")
                    nc.vector.reciprocal_approx_fast(rc32[:], den[:])
                    rc = mp.tile([1, QB], F32R, tag="rc", bufs=4,
                                 name=f"rc_{qb}_{h}")
                    nc.vector.tensor_copy(rc[:], rc32[:])
                    po_t[h] = (po, rc)

                def pass_a(h):
                    # diagonal tiles go first so tile tt=0 opens the full
                    # [0,512) PV accumulation region and the head's tail is
                    # short-latency.  PV lags the S/exp chain by one k-tile
                    # so the PE never sits waiting on the ACT exp.
                    full = [kt for kt in range(4 * qb)
                            if 128 * kt > QB * qb - WIN[h] - 127]
                    kts = list(range(4 * qb, 4 * qb + 4)) + full
                    po = pop.tile([D + 1, QB], F32, tag="po",
                                  name=f"po_{qb}_{h}")
                    pend = None
                    for i, kt in enumerate(kts):
                        tt = kt - 4 * qb
                        if tt >= 0:
                            c0, w = TRIM[tt]
                        else:
                            c0, w = 0, QB
                        pss = psp.tile([128, QB], F32, tag="ps")
                        nc.tensor.matmul(
                            pss[:, 0:w], ka[h][:, 128 * kt:128 * (kt + 1)],
                            qa_t[h][:, c0:c0 + w], start=True, stop=(tt < 0),
                            skip_group_check=True)
                        if tt >= 0:
                            # masked (k > q) entries get -3000 pre-exp
                            soff = 0 if tt == 3 else 128
                            nc.tensor.matmul(
                                pss[:, 0:w], ident_sb[:],
                                stair_sb[:, soff:soff + w], start=False,
                                stop=True, skip_group_check=True)
                        pt = ptp.tile([128, QB], F32R, tag="pt")
                        nc.scalar.activation(pt[:, 0:w], pss[:, 0:w], EXP,
                                             bias=0.0, scale=0.125)
                        if pend is not None:
                            pkt, pc0, pw, ppt = pend
                            nc.tensor.matmul(
                                po[:, pc0:pc0 + pw],
                                v_sb[pkt][:, 65 * h:65 * (h + 1)],
                                ppt[:, 0:pw], start=(i == 1), stop=False,
                                skip_group_check=True)
                        pend = (kt, c0, w, pt)
                    pkt, pc0, pw, ppt = pend
                    nc.tensor.matmul(
                        po[:, pc0:pc0 + pw],
                        v_sb[pkt][:, 65 * h:65 * (h + 1)], ppt[:, 0:pw],
                        start=(len(kts) == 1), stop=True,
                        skip_group_check=True)
                    finish_head(h, po)

                def pass_a23():
                    # slots 2,3 share one packed K/Q tile; common k-tiles
                    # issue as two concurrent row-tiled S matmuls.  PV lags
                    # by one k-tile so the PE never waits on the exp.
                    full2 = [kt for kt in range(4 * qb)
                             if 128 * kt > QB * qb - WIN[2] - 127]
                    kts = list(range(4 * qb, 4 * qb + 4)) + list(range(4 * qb))
                    po2 = pop.tile([D + 1, QB], F32, tag="po",
                                   name=f"po_{qb}_2")
                    po3 = pop.tile([D + 1, QB], F32, tag="po",
                                   name=f"po_{qb}_3")
                    n2 = 4 + len(full2)
                    n3 = len(kts)
                    i2 = i3 = 0

                    def pv_flush(pend, last):
                        nonlocal i2, i3
                        pkt, pc0, pw, pt2, pt3 = pend
                        if pt2 is not None:
                            nc.tensor.matmul(
                                po2[:, pc0:pc0 + pw],
                                v_sb[pkt][:, 65 * 2:65 * 3], pt2[:, 0:pw],
                                start=(i2 == 0), stop=(i2 == n2 - 1),
                                skip_group_check=True)
                            i2 += 1
                        nc.tensor.matmul(
                            po3[:, pc0:pc0 + pw],
                            v_sb[pkt][:, 65 * 3:65 * 4], pt3[:, 0:pw],
                            start=(i3 == 0), stop=(i3 == n3 - 1),
                            skip_group_check=True)
                        i3 += 1

                    pend = None
                    for kt in kts:
                        tt = kt - 4 * qb
                        if tt >= 0:
                            c0, w = TRIM[tt]
                        else:
                            c0, w = 0, QB
                        ktsl = slice(128 * kt, 128 * (kt + 1))
                        has2 = tt >= 0 or kt in full2
                        if has2:
                            pss2 = psp.tile([128, QB], F32, tag="ps")
                            nc.tensor.matmul(
                                pss2[:, 0:w], kap23[0:64, ktsl],
                                qa_t[2][0:64, c0:c0 + w], start=True,
                                stop=(tt < 0), skip_group_check=True,
                                tile_position=(0, 0))
                        pss3 = psp.tile([128, QB], F32, tag="ps")
                        nc.tensor.matmul(
                            pss3[:, 0:w], kap23[64:128, ktsl],
                            qa_t[2][64:128, c0:c0 + w], start=True,
                            stop=(tt < 0), skip_group_check=True,
                            tile_position=(64, 0))
                        if tt >= 0:
                            soff = 0 if tt == 3 else 128
                            nc.tensor.matmul(
                                pss2[:, 0:w], ident_sb[:],
                                stair_sb[:, soff:soff + w], start=False,
                                stop=True, skip_group_check=True)
                            nc.tensor.matmul(
                                pss3[:, 0:w], ident_sb[:],
                                stair_sb[:, soff:soff + w], start=False,
                                stop=True, skip_group_check=True)
                        pt2 = None
                        if has2:
                            bcol = 16 * qb + kt
                            pt2 = ptp.tile([128, QB], F32R, tag="pt")
                            nc.scalar.activation(
                                pt2[:, 0:w], pss2[:, 0:w], EXP,
                                bias=hb_sb[:, bcol:bcol + 1], scale=0.125)
                        bcol = 64 + 16 * qb + kt
                        pt3 = ptp.tile([128, QB], F32R, tag="pt")
                        nc.scalar.activation(
                            pt3[:, 0:w], pss3[:, 0:w], EXP,
                            bias=hb_sb[:, bcol:bcol + 1], scale=0.125)
                        if pend is not None:
                            pv_flush(pend, False)
                        pend = (kt, c0, w, pt2, pt3)
                    pv_flush(pend, True)
                    finish_head(3, po3)
                    finish_head(2, po2)

                def pass_b(h):
                    po, rc = po_t.pop(h)
                    pb = psp.tile([D, QB], F32, tag="ps",
                                  name=f"pb_{qb}_{h}")
                    nc.tensor.matmul(pb[:], ones_fr[:, 0:D], rc[:],
                                     start=True, stop=True,
                                     skip_group_check=True)
                    bc = mp.tile([D, QB], F32, tag="bc", bufs=4,
                                 name=f"bc_{qb}_{h}")
                    nc.vector.tensor_copy(bc[:], pb[:])
                    pair = ot_t[h // 2]
                    if h % 2 == 0:
                        nc.vector.tensor_tensor(pair[0:D, :], po[0:D, :],
                                                bc[:],
                                                op=mybir.AluOpType.mult)
                    else:
                        # odd head's O^T lands at partitions 0:64; DVE
                        # cannot shift partitions, so divide into a temp
                        # then DMA it into rows 64:128 of the pair tile
                        tmp = mp.tile([D, QB], BF16, tag="ottmp", bufs=4,
                                      name=f"ottmp_{qb}_{h}")
                        nc.vector.tensor_tensor(tmp[:], po[0:D, :], bc[:],
                                                op=mybir.AluOpType.mult)
                        # scalar HWDGE queue: keeps the Sync queue (which
                        # carries the xt prefetch) free of this hop
                        nc.scalar.dma_start(pair[D:2 * D, :], tmp[:])

                # Slots 2,3 (packed pair) first, then 1, then 0, so the qb's
                # trailing pass_b chain ends on even head 0 (no DMA hop); the
                # output projection starts on pair 1 (ready mid-sequence).
                pass_a23()
                pass_a(1)
                pass_b(3)
                pass_a(0)
                pass_b(2)

                # next q-block's projections are emitted BEFORE the last two
                # pass_b's and the output projection: the PE queue is
                # in-order, so these ready proj matmuls cover the ~3us DVE
                # recip/bcast chains of the trailing heads.
                if qb + 1 < NQB:
                    qa_next = proj(qb + 1)
                pass_b(1)
                pass_b(0)

                # output projection for this t-block (pair 1 first)
                for tt in range(4):
                    t = 4 * qb + tt
                    fsl = slice(128 * tt, 128 * (tt + 1))
                    ysb = ypool.tile([128, C], BF16, tag="y",
                                     name=f"y_{qb}_{tt}")
                    for half in range(2):
                        hsl = slice(QB * half, QB * (half + 1))
                        py = psp.tile([128, QB], F32, tag="ps")
                        for c in (1, 0):
                            nc.tensor.matmul(
                                py[:], ot_t[c][:, fsl], wo_sb[c][:, hsl],
                                start=(c == 1), stop=(c == 0),
                                skip_group_check=True)
                        nc.scalar.activation(ysb[:, hsl], py[:], CPY)
                        # sync queue: emitted after proj(qb+1)'s prefetch
                        # triggers, so these can't block the next q-block
                        nc.sync.dma_start(y[128 * t:128 * (t + 1), hsl],
                                          ysb[:, hsl])
    nc.finalize()
    return nc


_NC_CACHE = None


def _get_nc():
    global _NC_CACHE
    if _NC_CACHE is None:
        _NC_CACHE = _build()
    return _NC_CACHE


def kernel(x, Wq, bq, Wk, bk, Wv, bv, Wo, bo):
    x = np.asarray(x, dtype=np.float32)
    Wq, bq = np.asarray(Wq, np.float32), np.asarray(bq, np.float32)
    Wk, bk = np.asarray(Wk, np.float32), np.asarray(bk, np.float32)
    Wv, bv = np.asarray(Wv, np.float32), np.asarray(bv, np.float32)
    Wo, bo = np.asarray(Wo, np.float32), np.asarray(bo, np.float32)

    slopes = np.asarray(_slopes(H), dtype=np.float32)
    ar = np.arange(T, dtype=np.float32)

    pp, ff = np.meshgrid(np.arange(128), np.arange(640), indexing="ij")
    stair_np = np.where(ff - 128 < pp, -3000.0, 0.0).astype(BF)
    ident_np = np.eye(128, dtype=np.float32).astype(BF)

    xts = []
    for b in range(B):
        xa = np.empty((C + 1, T), np.float32)
        xa[:C] = x[b].T
        xa[C] = 1.0
        xts.append(np.ascontiguousarray(xa.astype(BF)))

    pr = np.arange(128, dtype=np.float32)
    shards = []
    for g in range(HG):
        # strided head assignment: core g, slot j <-> global head 4j+g, so
        # each slot's ALiBi slope range is uniform across cores and the
        # (SPMD-shared) graph can window steep slots' attention
        heads = [HG * j + g for j in range(HG)]
        # ACT-bias table for slots 2,3: col = 64*(slot-2) + 16*qb + kt,
        # value[p] = slope * (128*kt + p - 512*qb)
        hb = np.zeros((128, 128), np.float32)
        for sl in (2, 3):
            s = slopes[heads[sl]]
            for qbn in range(4):
                for kt in range(16):
                    col = 64 * (sl - 2) + 16 * qbn + kt
                    hb[:, col] = s * (128.0 * kt + pr - 512.0 * qbn)
        cols = np.concatenate([np.arange(D * h, D * (h + 1)) for h in heads])
        wqa = np.concatenate([Wq[:, cols], bq[None, cols]], axis=0)
        wka = np.concatenate([Wk[:, cols], bk[None, cols]], axis=0)
        wva = np.zeros((C + 1, VW), np.float32)
        for j, h in enumerate(heads):
            hsl = slice(D * h, D * (h + 1))
            wva[:C, 65 * j:65 * j + D] = Wv[:, hsl]
            wva[C, 65 * j:65 * j + D] = bv[hsl]
            wva[C, 65 * j + D] = 1.0
        woa = np.ascontiguousarray(Wo[cols, :])
        hk = np.empty((HG, 2, T), np.float32)
        hq = np.empty((HG, 2, T), np.float32)
        for j, h in enumerate(heads):
            # K rows (k, s8) pair with Q rows (s8, -q): S += s8*(k - q).
            # Integer k/q are exact on the f32r grid and s8 rounds once, so
            # the large terms cancel exactly in the fp32 PSUM accumulator
            # (splitting s8*k / s8*q would round each entry independently
            # and leave O(s8*T*eps) noise in the scores).
            s8 = 8.0 * slopes[h]
            hk[j, 0] = ar
            hk[j, 1] = s8
            hq[j, 0] = s8
            hq[j, 1] = -ar
        shards.append(dict(
            wq=np.ascontiguousarray(wqa.astype(BF)),
            wk=np.ascontiguousarray(wka.astype(BF)),
            wv=wva.astype(BF), wo=np.ascontiguousarray(woa.astype(BF)),
            hka=hk, hqa=hq, stair=stair_np, ident=ident_np, hbias=hb))

    in_maps = []
    for core in range(NCORES):
        b, g = divmod(core, HG)
        m = dict(shards[g])
        m["xt"] = xts[b]
        in_maps.append(m)

    nc = _get_nc()
    res = run_bass_kernel_spmd(nc, in_maps, core_ids=list(range(NCORES)))

    out = np.empty((B, T, C), np.float32)
    for b in range(B):
        acc = res.results[4 * b]["y"].astype(np.float32).copy()
        for g in range(1, HG):
            acc += res.results[4 * b + g]["y"].astype(np.float32)
        out[b] = acc + bo[None, :]
    return out

